# revision 1
# baseline (speedup 1.0000x reference)
"""MultiResolutionHashEncoding Trainium2 kernel.

Strategy (per NeuronCore, batch-sharded 8 ways):
  - Hash indices are computed on DVE with exact int math in the fp32-safe
    range (products kept < 2^24, primes pre-reduced mod 2^19).
  - The 2^19-entry tables are sliced into 16 slices of 2^15 entries; each
    GpSimd core's 16 partitions hold one full table copy (slice s on
    partition 16c+s) as scaled fp16 pairs.
  - ap_gather (per-core shared int16 index stream = low 15 bits) fetches the
    16 candidate entries per element; a second ap_gather into a tiny
    per-partition indicator table keyed by the high 4 bits produces a
    {0, 2^-13} mask (which also un-scales the fp16 table values).
  - DVE multiplies candidates by the mask; PE contracts each 16-partition
    group with a block-diagonal ones matrix; the selected values are DMA'd
    out in a strided pattern.
"""

import numpy as np

import concourse.tile_utils as tile_utils

tile_utils.max_sbuf_usage = 206 * 1024  # stale 192K default; cayman has 208K usable

import concourse.bacc as bacc
import concourse.tile as tile
import concourse.mybir as mybir
from concourse import bass
from concourse.bass_utils import run_bass_kernel_spmd

AluOp = mybir.AluOpType
dt = mybir.dt

N_LEVELS = 16
N_FEATS = 2
TABLE_SIZE = 524288  # 2**19
RESOLUTIONS = [16, 23, 32, 45, 64, 91, 128, 181, 256, 362, 512, 724, 1024,
               1448, 2048, 2896]
PRIMES = (1, 2654435761, 805459861)
BATCH = 2_097_152
N_CORES = 8

P = 128
BC = BATCH // N_CORES          # 262144 elements per core
SPP = BC // P                  # 2048 elements per partition
S_CHUNK = 128                  # s-range per processing chunk
N_CHUNKS = SPP // S_CHUNK      # 16 chunks per level
NI = 16 * S_CHUNK              # ap_gather num_idxs per core per chunk (2048)
SLICE = 32768                  # table entries per partition slice
SCALE = float(2 ** 13)         # table scaling for fp16 storage
MASK19 = 0x7FFFF

K1 = PRIMES[1] & MASK19        # 489905
K2 = PRIMES[2] & MASK19        # 95765

LAST_EXEC_SECONDS = None


def _emit_floor(nc, pool, src, R, out_dtype, tag, S):
    """g = floor(src * R) for src f32 [P, S]; exact w.r.t. f32 product."""
    v = pool.tile([P, S], dt.float32, tag="fl_v")
    nc.vector.tensor_scalar(v[:], src[:], float(R), None, AluOp.mult)
    r_i = pool.tile([P, S], dt.int32, tag="fl_ri")
    nc.vector.tensor_copy(r_i[:], v[:])          # round-to-nearest
    r_f = pool.tile([P, S], dt.float32, tag="fl_rf")
    nc.vector.tensor_copy(r_f[:], r_i[:])
    c = pool.tile([P, S], dt.float32, tag="fl_c")
    nc.vector.tensor_tensor(c[:], v[:], r_f[:], AluOp.is_lt)  # v < r_f -> 1.0
    g = pool.tile([P, S], out_dtype, tag=tag)
    nc.vector.tensor_tensor(g[:], r_f[:], c[:], AluOp.subtract)
    return g


def _emit_prime_mul(nc, pool, g_f, K, tag, S):
    """int32 tile whose low 19 bits equal (g*K) mod 2^19 (g < 4096)."""
    Khi, Klo = K >> 7, K & 127
    a = pool.tile([P, S], dt.int32, tag="pm_a")
    nc.vector.tensor_scalar(a[:], g_f[:], float(Khi), None, AluOp.mult)
    b = pool.tile([P, S], dt.int32, tag="pm_b")
    nc.vector.tensor_scalar(b[:], g_f[:], float(Klo), None, AluOp.mult)
    a0 = pool.tile([P, S], dt.int32, tag="pm_a0")
    nc.vector.tensor_scalar(a0[:], a[:], 0xFFF, None, AluOp.bitwise_and)
    comb = pool.tile([P, S], dt.int32, tag=tag)
    nc.vector.scalar_tensor_tensor(comb[:], a0[:], 128.0, b[:], AluOp.mult,
                                   AluOp.add)
    return comb


def build_nc():
    nc = bacc.Bacc(None, target_bir_lowering=False)

    # Per-core inputs: coords [3, P, SPP] f32, replicated tables
    # [N_LEVELS, P, SLICE, 2] f16 (pre-scaled by SCALE), block-diag ones.
    coords_in = nc.dram_tensor("coords3", [3, P, SPP], dt.float32,
                               kind="ExternalInput")
    tblr = nc.dram_tensor("tblr", [N_LEVELS, P, SLICE, N_FEATS], dt.float16,
                          kind="ExternalInput")
    b16_in = nc.dram_tensor("b16", [P, 8], dt.float16, kind="ExternalInput")
    ind_in = nc.dram_tensor("ind", [P, 16, 2], dt.float16,
                            kind="ExternalInput")
    out = nc.dram_tensor("out", [N_LEVELS, N_CHUNKS, 8, NI * N_FEATS],
                         dt.float32, kind="ExternalOutput")

    with tile.TileContext(nc) as tc:
        with (
            tc.tile_pool(name="tabp", bufs=1) as tabp,
            tc.tile_pool(name="workp", bufs=2) as workp,
            tc.tile_pool(name="hashp", bufs=1) as hashp,
            tc.tile_pool(name="constp", bufs=1) as constp,
            tc.tile_pool(name="psump", bufs=4, space="PSUM") as psump,
        ):
            b16 = constp.tile([P, 8], dt.float16, tag="b16")
            nc.sync.dma_start(b16[:], b16_in[:])
            ind = constp.tile([P, 16, 2], dt.float16, tag="ind")
            nc.sync.dma_start(ind[:], ind_in[:])
            mask19t = constp.tile([P, 1], dt.int32, tag="mask19t")
            nc.vector.memset(mask19t[:], MASK19)

            tabt = tabp.tile([P, SLICE, N_FEATS], dt.float16, tag="tabt")

            for lvl in range(N_LEVELS):
                R = RESOLUTIONS[lvl]
                nc.sync.dma_start(tabt[:], tblr[lvl])
                for ch in range(N_CHUNKS):
                    s0 = ch * S_CHUNK
                    sl = slice(s0, s0 + S_CHUNK)
                    # --- load coords chunk
                    xt = hashp.tile([P, S_CHUNK], dt.float32, tag="xt")
                    yt = hashp.tile([P, S_CHUNK], dt.float32, tag="yt")
                    zt = hashp.tile([P, S_CHUNK], dt.float32, tag="zt")
                    nc.sync.dma_start(xt[:], coords_in[0, :, sl])
                    nc.sync.dma_start(yt[:], coords_in[1, :, sl])
                    nc.sync.dma_start(zt[:], coords_in[2, :, sl])

                    # --- hash
                    gx = _emit_floor(nc, hashp, xt, R, dt.int32, "gx", S_CHUNK)
                    gy = _emit_floor(nc, hashp, yt, R, dt.float32, "gy", S_CHUNK)
                    gz = _emit_floor(nc, hashp, zt, R, dt.float32, "gz", S_CHUNK)
                    py_ = _emit_prime_mul(nc, hashp, gy, K1, "py", S_CHUNK)
                    pz_ = _emit_prime_mul(nc, hashp, gz, K2, "pz", S_CHUNK)
                    t1 = hashp.tile([P, S_CHUNK], dt.int32, tag="t1")
                    nc.vector.scalar_tensor_tensor(
                        t1[:], py_[:], mask19t[:], gx[:],
                        AluOp.bitwise_and, AluOp.bitwise_xor)
                    h = hashp.tile([P, S_CHUNK], dt.int32, tag="h")
                    nc.vector.scalar_tensor_tensor(
                        h[:], pz_[:], mask19t[:], t1[:],
                        AluOp.bitwise_and, AluOp.bitwise_xor)
                    lo32 = hashp.tile([P, S_CHUNK], dt.int32, tag="lo32")
                    nc.vector.tensor_scalar(lo32[:], h[:], 0x7FFF, None,
                                            AluOp.bitwise_and)
                    lo = hashp.tile([P, S_CHUNK], dt.int16, tag="lo")
                    nc.vector.tensor_copy(lo[:], lo32[:])
                    hi32 = hashp.tile([P, S_CHUNK], dt.int32, tag="hi32")
                    nc.vector.tensor_scalar(hi32[:], h[:], 15, None,
                                            AluOp.logical_shift_right)
                    hi = hashp.tile([P, S_CHUNK], dt.int16, tag="hi")
                    nc.vector.tensor_copy(hi[:], hi32[:])

                    # --- gathers
                    cand = workp.tile([P, NI, N_FEATS], dt.float16, tag="cand")
                    nc.gpsimd.ap_gather(cand[:], tabt[:], lo[:], channels=P,
                                        num_elems=SLICE, d=N_FEATS,
                                        num_idxs=NI)
                    maskt = workp.tile([P, NI, N_FEATS], dt.float16, tag="maskt")
                    nc.gpsimd.ap_gather(maskt[:], ind[:], hi[:], channels=P,
                                        num_elems=16, d=N_FEATS, num_idxs=NI)

                    # --- select: mask-mult in place, block-sum on PE,
                    # descale during PSUM evacuation
                    cfl = cand[:].rearrange("p n f -> p (n f)")
                    mfl = maskt[:].rearrange("p n f -> p (n f)")
                    nc.vector.tensor_tensor(cfl, cfl, mfl, AluOp.mult)
                    sel = workp.tile([8, NI * N_FEATS], dt.float32, tag="sel")
                    NCOL = 512
                    for mcol in range(0, NI * N_FEATS, NCOL):
                        ps = psump.tile([8, NCOL], dt.float32, space="PSUM",
                                        tag="ps")
                        nc.tensor.matmul(ps[:], b16[:],
                                         cfl[:, mcol:mcol + NCOL],
                                         start=True, stop=True)
                        nc.vector.tensor_scalar(sel[:, mcol:mcol + NCOL],
                                                ps[:], 1.0 / SCALE,
                                                None, AluOp.mult)

                    # --- store raw; host unscrambles (sj, q, f) interleave
                    nc.sync.dma_start(out[lvl, ch], sel[:])

    nc.compile()
    return nc


def _prep_tables(tables):
    """[L, T, F] f32 -> replicated sliced fp16 [L, P, SLICE, F] (scaled)."""
    t16 = (tables * SCALE).astype(np.float16)  # [L, T, F]
    sl = t16.reshape(N_LEVELS, 16, SLICE, N_FEATS)  # slice s = entries s*SLICE+
    # partition p holds slice p % 16
    return np.ascontiguousarray(sl[:, np.arange(P) % 16])  # [L, P, SLICE, F]


def kernel(coords, tables):
    global LAST_EXEC_SECONDS
    coords = np.asarray(coords, dtype=np.float32)
    tables = np.asarray(tables, dtype=np.float32)

    tblr = _prep_tables(tables)
    b16 = np.zeros((P, 8), np.float16)
    for g in range(8):
        b16[g * 16:(g + 1) * 16, g] = 1.0
    ind = np.zeros((P, 16, N_FEATS), np.float16)
    for p in range(P):
        ind[p, p % 16, :] = np.float16(1.0)

    nc = build_nc()

    in_maps = []
    for c in range(N_CORES):
        csl = coords[c * BC:(c + 1) * BC]  # [BC, 3]
        c3 = np.ascontiguousarray(csl.T.reshape(3, P, SPP))
        in_maps.append({"coords3": c3, "tblr": tblr, "b16": b16, "ind": ind})

    import time
    t0 = time.time()
    res = run_bass_kernel_spmd(nc, in_maps, core_ids=list(range(N_CORES)))
    LAST_EXEC_SECONDS = time.time() - t0

    out = np.empty((BATCH, N_LEVELS * N_FEATS), np.float32)
    n_lvl = len(RESOLUTIONS[:N_LEVELS])
    for c in range(N_CORES):
        oc = res.results[c]["out"]  # [L, NCH, 8, NI*F]
        oc = oc.reshape(n_lvl, N_CHUNKS, 8, S_CHUNK, 16, N_FEATS)
        # axes (l, ch, g, sj, q, f) -> b = ((g*16+q)*SPP + ch*S_CHUNK + sj)
        oc = oc.transpose(2, 4, 1, 3, 0, 5).reshape(BC, n_lvl * N_FEATS)
        out[c * BC:(c + 1) * BC] = oc
    return out



# revision 3
# speedup vs baseline: 3737.7641x; 3737.7641x over previous
"""MultiResolutionHashEncoding Trainium2 kernel.

Strategy (per NeuronCore, batch-sharded 8 ways):
  - Hash indices are computed on DVE with exact int math in the fp32-safe
    range (products kept < 2^24, primes pre-reduced mod 2^19).
  - The 2^19-entry tables are sliced into 16 slices of 2^15 entries; each
    GpSimd core's 16 partitions hold one full table copy (slice s on
    partition 16c+s) as scaled fp16 pairs.
  - ap_gather (per-core shared int16 index stream = low 15 bits) fetches the
    16 candidate entries per element; a second ap_gather into a tiny
    per-partition indicator table keyed by the high 4 bits produces a
    {0, 2^-13} mask (which also un-scales the fp16 table values).
  - DVE multiplies candidates by the mask; PE contracts each 16-partition
    group with a block-diagonal ones matrix; the selected values are DMA'd
    out in a strided pattern.
"""

import numpy as np

import concourse.tile_utils as tile_utils

tile_utils.max_sbuf_usage = 206 * 1024  # stale 192K default; cayman has 208K usable

import concourse.bacc as bacc
import concourse.tile as tile
import concourse.mybir as mybir
from concourse import bass
from concourse.bass_utils import run_bass_kernel_spmd

AluOp = mybir.AluOpType
dt = mybir.dt

N_LEVELS = 16
N_FEATS = 2
TABLE_SIZE = 524288  # 2**19
RESOLUTIONS = [16, 23, 32, 45, 64, 91, 128, 181, 256, 362, 512, 724, 1024,
               1448, 2048, 2896]
PRIMES = (1, 2654435761, 805459861)
BATCH = 2_097_152
N_CORES = 8

P = 128
BC = BATCH // N_CORES          # 262144 elements per core
SPP = BC // P                  # 2048 elements per partition
S_CHUNK = 128                  # s-range per processing chunk
N_CHUNKS = SPP // S_CHUNK      # 16 chunks per level
NI = 16 * S_CHUNK              # ap_gather num_idxs per core per chunk (2048)
SLICE = 32768                  # table entries per partition slice
SCALE = float(2 ** 13)         # table scaling for fp16 storage
MASK19 = 0x7FFFF

K1 = PRIMES[1] & MASK19        # 489905
K2 = PRIMES[2] & MASK19        # 95765

LAST_EXEC_SECONDS = None
LAST_EXEC_NS = None
LAST_TRACE = None


def _emit_floor(nc, pool, src, R, out_dtype, tag, S):
    """g = floor(src * R) for src f32 [P, S]; exact w.r.t. f32 product."""
    v = pool.tile([P, S], dt.float32, tag="fl_v")
    nc.vector.tensor_scalar(v[:], src[:], float(R), None, AluOp.mult)
    r_i = pool.tile([P, S], dt.int32, tag="fl_ri")
    nc.vector.tensor_copy(r_i[:], v[:])          # round-to-nearest
    r_f = pool.tile([P, S], dt.float32, tag="fl_rf")
    nc.vector.tensor_copy(r_f[:], r_i[:])
    c = pool.tile([P, S], dt.float32, tag="fl_c")
    nc.vector.tensor_tensor(c[:], v[:], r_f[:], AluOp.is_lt)  # v < r_f -> 1.0
    g = pool.tile([P, S], out_dtype, tag=tag)
    nc.vector.tensor_tensor(g[:], r_f[:], c[:], AluOp.subtract)
    return g


def _emit_prime_mul(nc, pool, g_f, K, tag, S):
    """int32 tile whose low 19 bits equal (g*K) mod 2^19 (g < 4096)."""
    Khi, Klo = K >> 7, K & 127
    a = pool.tile([P, S], dt.int32, tag="pm_a")
    nc.vector.tensor_scalar(a[:], g_f[:], float(Khi), None, AluOp.mult)
    b = pool.tile([P, S], dt.int32, tag="pm_b")
    nc.vector.tensor_scalar(b[:], g_f[:], float(Klo), None, AluOp.mult)
    a0 = pool.tile([P, S], dt.int32, tag="pm_a0")
    nc.vector.tensor_scalar(a0[:], a[:], 0xFFF, None, AluOp.bitwise_and)
    comb = pool.tile([P, S], dt.int32, tag=tag)
    nc.vector.scalar_tensor_tensor(comb[:], a0[:], 128.0, b[:], AluOp.mult,
                                   AluOp.add)
    return comb


def build_nc():
    nc = bacc.Bacc(None, target_bir_lowering=False)

    # Per-core inputs: coords [3, P, SPP] f32, replicated tables
    # [N_LEVELS, P, SLICE, 2] f16 (pre-scaled by SCALE), block-diag ones.
    coords_in = nc.dram_tensor("coords3", [3, P, SPP], dt.float32,
                               kind="ExternalInput")
    tblr = nc.dram_tensor("tblr", [N_LEVELS, P, SLICE, N_FEATS], dt.float16,
                          kind="ExternalInput")
    b16_in = nc.dram_tensor("b16", [P, 8], dt.float16, kind="ExternalInput")
    ind_in = nc.dram_tensor("ind", [P, 16, 2], dt.float16,
                            kind="ExternalInput")
    out = nc.dram_tensor("out", [N_LEVELS, N_CHUNKS, 8, NI * N_FEATS],
                         dt.float32, kind="ExternalOutput")

    with tile.TileContext(nc) as tc:
        with (
            tc.tile_pool(name="tabp", bufs=1) as tabp,
            tc.tile_pool(name="workp", bufs=2) as workp,
            tc.tile_pool(name="hashp", bufs=1) as hashp,
            tc.tile_pool(name="constp", bufs=1) as constp,
            tc.tile_pool(name="psump", bufs=4, space="PSUM") as psump,
        ):
            b16 = constp.tile([P, 8], dt.float16, tag="b16")
            nc.sync.dma_start(b16[:], b16_in[:])
            ind = constp.tile([P, 16, 2], dt.float16, tag="ind")
            nc.sync.dma_start(ind[:], ind_in[:])
            mask19t = constp.tile([P, 1], dt.int32, tag="mask19t")
            nc.vector.memset(mask19t[:], MASK19)

            tabt = tabp.tile([P, SLICE, N_FEATS], dt.float16, tag="tabt")

            for lvl in range(N_LEVELS):
                R = RESOLUTIONS[lvl]
                nc.sync.dma_start(tabt[:], tblr[lvl])
                for ch in range(N_CHUNKS):
                    s0 = ch * S_CHUNK
                    sl = slice(s0, s0 + S_CHUNK)
                    # --- load coords chunk
                    xt = hashp.tile([P, S_CHUNK], dt.float32, tag="xt")
                    yt = hashp.tile([P, S_CHUNK], dt.float32, tag="yt")
                    zt = hashp.tile([P, S_CHUNK], dt.float32, tag="zt")
                    nc.sync.dma_start(xt[:], coords_in[0, :, sl])
                    nc.sync.dma_start(yt[:], coords_in[1, :, sl])
                    nc.sync.dma_start(zt[:], coords_in[2, :, sl])

                    # --- hash
                    gx = _emit_floor(nc, hashp, xt, R, dt.int32, "gx", S_CHUNK)
                    gy = _emit_floor(nc, hashp, yt, R, dt.float32, "gy", S_CHUNK)
                    gz = _emit_floor(nc, hashp, zt, R, dt.float32, "gz", S_CHUNK)
                    py_ = _emit_prime_mul(nc, hashp, gy, K1, "py", S_CHUNK)
                    pz_ = _emit_prime_mul(nc, hashp, gz, K2, "pz", S_CHUNK)
                    t1 = hashp.tile([P, S_CHUNK], dt.int32, tag="t1")
                    nc.vector.scalar_tensor_tensor(
                        t1[:], py_[:], mask19t[:], gx[:],
                        AluOp.bitwise_and, AluOp.bitwise_xor)
                    h = hashp.tile([P, S_CHUNK], dt.int32, tag="h")
                    nc.vector.scalar_tensor_tensor(
                        h[:], pz_[:], mask19t[:], t1[:],
                        AluOp.bitwise_and, AluOp.bitwise_xor)
                    lo32 = hashp.tile([P, S_CHUNK], dt.int32, tag="lo32")
                    nc.vector.tensor_scalar(lo32[:], h[:], 0x7FFF, None,
                                            AluOp.bitwise_and)
                    lo = hashp.tile([P, S_CHUNK], dt.int16, tag="lo")
                    nc.vector.tensor_copy(lo[:], lo32[:])
                    hi32 = hashp.tile([P, S_CHUNK], dt.int32, tag="hi32")
                    nc.vector.tensor_scalar(hi32[:], h[:], 15, None,
                                            AluOp.logical_shift_right)
                    hi = hashp.tile([P, S_CHUNK], dt.int16, tag="hi")
                    nc.vector.tensor_copy(hi[:], hi32[:])

                    # --- gathers
                    cand = workp.tile([P, NI, N_FEATS], dt.float16, tag="cand")
                    nc.gpsimd.ap_gather(cand[:], tabt[:], lo[:], channels=P,
                                        num_elems=SLICE, d=N_FEATS,
                                        num_idxs=NI)
                    maskt = workp.tile([P, NI, N_FEATS], dt.float16, tag="maskt")
                    nc.gpsimd.ap_gather(maskt[:], ind[:], hi[:], channels=P,
                                        num_elems=16, d=N_FEATS, num_idxs=NI)

                    # --- select: mask-mult in place, block-sum on PE,
                    # descale during PSUM evacuation
                    cfl = cand[:].rearrange("p n f -> p (n f)")
                    mfl = maskt[:].rearrange("p n f -> p (n f)")
                    nc.vector.tensor_tensor(cfl, cfl, mfl, AluOp.mult)
                    sel = workp.tile([8, NI * N_FEATS], dt.float32, tag="sel")
                    NCOL = 512
                    for mcol in range(0, NI * N_FEATS, NCOL):
                        ps = psump.tile([8, NCOL], dt.float32, space="PSUM",
                                        tag="ps")
                        nc.tensor.matmul(ps[:], b16[:],
                                         cfl[:, mcol:mcol + NCOL],
                                         start=True, stop=True)
                        nc.vector.tensor_scalar(sel[:, mcol:mcol + NCOL],
                                                ps[:], 1.0 / SCALE,
                                                None, AluOp.mult)

                    # --- store raw; host unscrambles (sj, q, f) interleave
                    nc.sync.dma_start(out[lvl, ch], sel[:])

    nc.compile()
    return nc


def _prep_tables(tables):
    """[L, T, F] f32 -> replicated sliced fp16 [L, P, SLICE, F] (scaled)."""
    t16 = (tables * SCALE).astype(np.float16)  # [L, T, F]
    sl = t16.reshape(N_LEVELS, 16, SLICE, N_FEATS)  # slice s = entries s*SLICE+
    # partition p holds slice p % 16
    return np.ascontiguousarray(sl[:, np.arange(P) % 16])  # [L, P, SLICE, F]


def kernel(coords, tables):
    global LAST_EXEC_SECONDS
    coords = np.asarray(coords, dtype=np.float32)
    tables = np.asarray(tables, dtype=np.float32)

    tblr = _prep_tables(tables)
    b16 = np.zeros((P, 8), np.float16)
    for g in range(8):
        b16[g * 16:(g + 1) * 16, g] = 1.0
    ind = np.zeros((P, 16, N_FEATS), np.float16)
    for p in range(P):
        ind[p, p % 16, :] = np.float16(1.0)

    nc = build_nc()

    in_maps = []
    for c in range(N_CORES):
        csl = coords[c * BC:(c + 1) * BC]  # [BC, 3]
        c3 = np.ascontiguousarray(csl.T.reshape(3, P, SPP))
        in_maps.append({"coords3": c3, "tblr": tblr, "b16": b16, "ind": ind})

    import time
    global LAST_EXEC_NS, LAST_TRACE
    t0 = time.time()
    res = run_bass_kernel_spmd(nc, in_maps, core_ids=list(range(N_CORES)))
    LAST_EXEC_SECONDS = time.time() - t0
    LAST_EXEC_NS = getattr(res, "exec_time_ns", None)
    LAST_TRACE = getattr(res, "instructions_and_trace", None)

    out = np.empty((BATCH, N_LEVELS * N_FEATS), np.float32)
    n_lvl = len(RESOLUTIONS[:N_LEVELS])
    for c in range(N_CORES):
        oc = res.results[c]["out"]  # [L, NCH, 8, NI*F]
        oc = oc.reshape(n_lvl, N_CHUNKS, 8, S_CHUNK, 16, N_FEATS)
        # axes (l, ch, g, sj, q, f) -> b = ((g*16+q)*SPP + ch*S_CHUNK + sj)
        oc = oc.transpose(2, 4, 1, 3, 0, 5).reshape(BC, n_lvl * N_FEATS)
        out[c * BC:(c + 1) * BC] = oc
    return out



# revision 17
# speedup vs baseline: 6758.0016x; 1.8080x over previous
"""MultiResolutionHashEncoding Trainium2 kernel.

Strategy (per NeuronCore, batch-sharded 8 ways):
  - Hash indices are computed on DVE with exact int math in the fp32-safe
    range (products kept < 2^24, primes pre-reduced mod 2^19).
  - The 2^19-entry tables are sliced into 16 slices of 2^15 entries; each
    GpSimd core's 16 partitions hold one full table copy (slice s on
    partition 16c+s) as scaled fp16 pairs.
  - ap_gather (per-core shared int16 index stream = low 15 bits) fetches the
    16 candidate entries per element.
  - The slice-select mask is computed without a second gather: 16 selector
    matmuls on the (idle) PE broadcast each element's high-4 hash bits to
    all 16 partitions of its group, and DVE is_equal against the partition's
    slice id yields the {0,1} fp16 mask.
  - DVE multiplies candidates by the mask; PE contracts each 16-partition
    group with a block-diagonal ones matrix; the selected values are DMA'd
    out in a strided pattern.
"""

import numpy as np

import concourse.tile_utils as tile_utils

tile_utils.max_sbuf_usage = 206 * 1024  # stale 192K default; cayman has 208K usable

import concourse.bacc as bacc
import concourse.tile as tile
import concourse.mybir as mybir
from concourse import bass
from concourse.bass_utils import run_bass_kernel_spmd

AluOp = mybir.AluOpType
dt = mybir.dt

N_LEVELS = 16
N_FEATS = 2
TABLE_SIZE = 524288  # 2**19
RESOLUTIONS = [16, 23, 32, 45, 64, 91, 128, 181, 256, 362, 512, 724, 1024,
               1448, 2048, 2896]
PRIMES = (1, 2654435761, 805459861)
BATCH = 2_097_152
N_CORES = 8

P = 128
BC = BATCH // N_CORES          # 262144 elements per core
SPP = BC // P                  # 2048 elements per partition
S_CHUNK = 128                  # s-range per processing chunk
N_CHUNKS = SPP // S_CHUNK      # 16 chunks per level
NI = 16 * S_CHUNK              # ap_gather num_idxs per core per chunk (2048)
SLICE = 32768                  # table entries per partition slice
SCALE = float(2 ** 13)         # table scaling for fp16 storage
MASK19 = 0x7FFFF

K1 = PRIMES[1] & MASK19        # 489905
K2 = PRIMES[2] & MASK19        # 95765

LAST_EXEC_SECONDS = None
LAST_EXEC_NS = None
LAST_TRACE = None


def _emit_floor(nc, pool, src, R, out_dtype, tag, S):
    """g = floor(src * R) for src f32 [P, S]; exact w.r.t. f32 product."""
    v = pool.tile([P, S], dt.float32, tag="fl_v")
    nc.vector.tensor_scalar(v[:], src[:], float(R), None, AluOp.mult)
    r_i = pool.tile([P, S], dt.int32, tag="fl_ri")
    nc.vector.tensor_copy(r_i[:], v[:])          # round-to-nearest
    r_f = pool.tile([P, S], dt.float32, tag="fl_rf")
    nc.vector.tensor_copy(r_f[:], r_i[:])
    c = pool.tile([P, S], dt.float32, tag="fl_c")
    nc.vector.tensor_tensor(c[:], v[:], r_f[:], AluOp.is_lt)  # v < r_f -> 1.0
    g = pool.tile([P, S], out_dtype, tag=tag)
    nc.vector.tensor_tensor(g[:], r_f[:], c[:], AluOp.subtract)
    return g


def _emit_prime_mul(nc, pool, g_f, K, tag, S):
    """int32 tile whose low 19 bits equal (g*K) mod 2^19 (g < 4096)."""
    Khi, Klo = K >> 7, K & 127
    a = pool.tile([P, S], dt.int32, tag="pm_a")
    nc.vector.tensor_scalar(a[:], g_f[:], float(Khi), None, AluOp.mult)
    b = pool.tile([P, S], dt.int32, tag="pm_b")
    nc.vector.tensor_scalar(b[:], g_f[:], float(Klo), None, AluOp.mult)
    a0 = pool.tile([P, S], dt.int32, tag="pm_a0")
    nc.vector.tensor_scalar(a0[:], a[:], 0xFFF, None, AluOp.bitwise_and)
    comb = pool.tile([P, S], dt.int32, tag=tag)
    nc.vector.scalar_tensor_tensor(comb[:], a0[:], 128.0, b[:], AluOp.mult,
                                   AluOp.add)
    return comb


def build_nc():
    nc = bacc.Bacc(None, target_bir_lowering=False)

    # Per-core inputs: coords [3, P, SPP] f32, replicated tables
    # [N_LEVELS, P, SLICE, 2] f16 (pre-scaled by SCALE), block-diag ones.
    coords_in = nc.dram_tensor("coords3", [3, P, SPP], dt.float32,
                               kind="ExternalInput")
    tblr = nc.dram_tensor("tblr", [N_LEVELS, P, SLICE, N_FEATS], dt.float16,
                          kind="ExternalInput")
    b16_in = nc.dram_tensor("b16", [P, 8], dt.float16, kind="ExternalInput")
    wq_in = nc.dram_tensor("wq", [P, 16, S_CHUNK], dt.float16,
                           kind="ExternalInput")
    sid_in = nc.dram_tensor("sid", [P, 1], dt.float32, kind="ExternalInput")
    out = nc.dram_tensor("out", [N_LEVELS, N_CHUNKS, 8, NI * N_FEATS],
                         dt.float32, kind="ExternalOutput")

    with tile.TileContext(nc) as tc:
        with (
            tc.tile_pool(name="tabp", bufs=1) as tabp,
            tc.tile_pool(name="workp", bufs=2) as workp,
            tc.tile_pool(name="hashp", bufs=1) as hashp,
            tc.tile_pool(name="iop", bufs=2) as iop,
            tc.tile_pool(name="coop", bufs=2) as coop,
            tc.tile_pool(name="constp", bufs=1) as constp,
            tc.tile_pool(name="psump", bufs=4, space="PSUM") as psump,
            tc.tile_pool(name="psumq", bufs=4, space="PSUM") as psumq,
        ):
            b16 = constp.tile([P, 8], dt.float16, tag="b16")
            nc.sync.dma_start(b16[:], b16_in[:])
            wq = constp.tile([P, 16, S_CHUNK], dt.float16, tag="wq")
            nc.sync.dma_start(wq[:], wq_in[:])
            sid = constp.tile([P, 1], dt.float32, tag="sid")
            nc.sync.dma_start(sid[:], sid_in[:])
            mask19t = constp.tile([P, 1], dt.int32, tag="mask19t")
            nc.vector.memset(mask19t[:], MASK19)

            tabt = tabp.tile([P, SLICE, N_FEATS], dt.float16, tag="tabt")

            for lvl in range(N_LEVELS):
                R = RESOLUTIONS[lvl]
                nc.sync.dma_start(tabt[:], tblr[lvl])
                for ch in range(N_CHUNKS):
                    s0 = ch * S_CHUNK
                    sl = slice(s0, s0 + S_CHUNK)
                    # --- load coords chunk
                    xt = coop.tile([P, S_CHUNK], dt.float32, tag="xt")
                    yt = coop.tile([P, S_CHUNK], dt.float32, tag="yt")
                    zt = coop.tile([P, S_CHUNK], dt.float32, tag="zt")
                    nc.sync.dma_start(xt[:], coords_in[0, :, sl])
                    nc.sync.dma_start(yt[:], coords_in[1, :, sl])
                    nc.sync.dma_start(zt[:], coords_in[2, :, sl])

                    # --- hash
                    gx = _emit_floor(nc, hashp, xt, R, dt.int32, "gx", S_CHUNK)
                    gy = _emit_floor(nc, hashp, yt, R, dt.float32, "gy", S_CHUNK)
                    gz = _emit_floor(nc, hashp, zt, R, dt.float32, "gz", S_CHUNK)
                    py_ = _emit_prime_mul(nc, hashp, gy, K1, "py", S_CHUNK)
                    pz_ = _emit_prime_mul(nc, hashp, gz, K2, "pz", S_CHUNK)
                    # scratch tags pm_a/pm_b/pm_a0 are dead by now; alias them
                    t1 = hashp.tile([P, S_CHUNK], dt.int32, tag="pm_a")
                    nc.vector.scalar_tensor_tensor(
                        t1[:], py_[:], mask19t[:], gx[:],
                        AluOp.bitwise_and, AluOp.bitwise_xor)
                    h = hashp.tile([P, S_CHUNK], dt.int32, tag="pm_b")
                    nc.vector.scalar_tensor_tensor(
                        h[:], pz_[:], mask19t[:], t1[:],
                        AluOp.bitwise_and, AluOp.bitwise_xor)
                    lo32 = hashp.tile([P, S_CHUNK], dt.int32, tag="pm_a0")
                    nc.vector.tensor_scalar(lo32[:], h[:], 0x7FFF, None,
                                            AluOp.bitwise_and)
                    lo = iop.tile([P, S_CHUNK], dt.int16, tag="lo")
                    nc.vector.tensor_copy(lo[:], lo32[:])
                    hi32 = hashp.tile([P, S_CHUNK], dt.int32, tag="pm_a")
                    nc.vector.tensor_scalar(hi32[:], h[:], 15, None,
                                            AluOp.logical_shift_right)
                    hi_f = iop.tile([P, S_CHUNK], dt.float16, tag="hi_f")
                    nc.vector.tensor_copy(hi_f[:], hi32[:])

                    # --- candidate gather
                    cand = workp.tile([P, NI, N_FEATS], dt.float16, tag="cand")
                    nc.gpsimd.ap_gather(cand[:], tabt[:], lo[:], channels=P,
                                        num_elems=SLICE, d=N_FEATS,
                                        num_idxs=NI)

                    # --- slice mask: broadcast hi across each 16-partition
                    # group via selector matmuls, compare against slice id
                    maskt = workp.tile([P, NI, N_FEATS], dt.float16, tag="maskt")
                    for q in range(16):
                        psq = psumq.tile([P, S_CHUNK], dt.float32,
                                         space="PSUM", tag="psq")
                        nc.tensor.matmul(psq[:], wq[:, q], hi_f[:],
                                         start=True, stop=True)
                        psb = psq[:].unsqueeze(2).broadcast_to(
                            [P, S_CHUNK, N_FEATS])
                        nc.vector.tensor_scalar(
                            maskt[:, q::16, :], psb, sid[:], None,
                            AluOp.is_equal)

                    # --- select: mask-mult in place, block-sum on PE,
                    # descale during PSUM evacuation
                    cfl = cand[:].rearrange("p n f -> p (n f)")
                    mfl = maskt[:].rearrange("p n f -> p (n f)")
                    nc.vector.tensor_tensor(cfl, cfl, mfl, AluOp.mult)
                    sel = workp.tile([8, NI * N_FEATS], dt.float32, tag="sel")
                    NCOL = 512
                    for mcol in range(0, NI * N_FEATS, NCOL):
                        ps = psump.tile([8, NCOL], dt.float32, space="PSUM",
                                        tag="ps")
                        nc.tensor.matmul(ps[:], b16[:],
                                         cfl[:, mcol:mcol + NCOL],
                                         start=True, stop=True)
                        nc.vector.tensor_scalar(sel[:, mcol:mcol + NCOL],
                                                ps[:], 1.0 / SCALE,
                                                None, AluOp.mult)

                    # --- store raw; host unscrambles (sj, q, f) interleave
                    nc.sync.dma_start(out[lvl, ch], sel[:])

    nc.compile()
    return nc


def _prep_tables(tables):
    """[L, T, F] f32 -> replicated sliced fp16 [L, P, SLICE, F] (scaled)."""
    t16 = (tables * SCALE).astype(np.float16)  # [L, T, F]
    sl = t16.reshape(N_LEVELS, 16, SLICE, N_FEATS)  # slice s = entries s*SLICE+
    # partition p holds slice p % 16
    return np.ascontiguousarray(sl[:, np.arange(P) % 16])  # [L, P, SLICE, F]


def kernel(coords, tables):
    global LAST_EXEC_SECONDS
    coords = np.asarray(coords, dtype=np.float32)
    tables = np.asarray(tables, dtype=np.float32)

    tblr = _prep_tables(tables)
    b16 = np.zeros((P, 8), np.float16)
    for g in range(8):
        b16[g * 16:(g + 1) * 16, g] = 1.0
    # selector weights: wq[p, q, c] = 1 iff p == (c//16)*16 + q
    wq = np.zeros((P, 16, S_CHUNK), np.float16)
    for q in range(16):
        for c in range(S_CHUNK):
            wq[(c // 16) * 16 + q, q, c] = 1.0
    sid = (np.arange(P) % 16).astype(np.float32).reshape(P, 1)

    nc = build_nc()

    in_maps = []
    for c in range(N_CORES):
        csl = coords[c * BC:(c + 1) * BC]  # [BC, 3]
        c3 = np.ascontiguousarray(csl.T.reshape(3, P, SPP))
        in_maps.append({"coords3": c3, "tblr": tblr, "b16": b16, "wq": wq,
                        "sid": sid})

    import time
    global LAST_EXEC_NS, LAST_TRACE
    t0 = time.time()
    res = run_bass_kernel_spmd(nc, in_maps, core_ids=list(range(N_CORES)))
    LAST_EXEC_SECONDS = time.time() - t0
    LAST_EXEC_NS = getattr(res, "exec_time_ns", None)
    LAST_TRACE = getattr(res, "instructions_and_trace", None)

    out = np.empty((BATCH, N_LEVELS * N_FEATS), np.float32)
    n_lvl = len(RESOLUTIONS[:N_LEVELS])
    for c in range(N_CORES):
        oc = res.results[c]["out"]  # [L, NCH, 8, NI*F]
        oc = oc.reshape(n_lvl, N_CHUNKS, 8, S_CHUNK, 16, N_FEATS)
        # axes (l, ch, g, sj, q, f) -> b = ((g*16+q)*SPP + ch*S_CHUNK + sj)
        oc = oc.transpose(2, 4, 1, 3, 0, 5).reshape(BC, n_lvl * N_FEATS)
        out[c * BC:(c + 1) * BC] = oc
    return out



# revision 18
# speedup vs baseline: 6811.7923x; 1.0080x over previous
"""MultiResolutionHashEncoding Trainium2 kernel.

Strategy (per NeuronCore, batch-sharded 8 ways):
  - Hash indices are computed on DVE with exact int math in the fp32-safe
    range (products kept < 2^24, primes pre-reduced mod 2^19).
  - The 2^19-entry tables are sliced into 16 slices of 2^15 entries; each
    GpSimd core's 16 partitions hold one full table copy (slice s on
    partition 16c+s) as scaled fp16 pairs.
  - ap_gather (per-core shared int16 index stream = low 15 bits) fetches the
    16 candidate entries per element.
  - The slice-select mask is computed without a second gather: 16 selector
    matmuls on the (idle) PE broadcast each element's high-4 hash bits to
    all 16 partitions of its group, and DVE is_equal against the partition's
    slice id yields the {0,1} fp16 mask.
  - DVE multiplies candidates by the mask; PE contracts each 16-partition
    group with a block-diagonal ones matrix; the selected values are DMA'd
    out in a strided pattern.
"""

import numpy as np

import concourse.tile_utils as tile_utils

tile_utils.max_sbuf_usage = 206 * 1024  # stale 192K default; cayman has 208K usable

import concourse.bacc as bacc
import concourse.tile as tile
import concourse.mybir as mybir
from concourse import bass
from concourse.bass_utils import run_bass_kernel_spmd

AluOp = mybir.AluOpType
dt = mybir.dt

N_LEVELS = 16
N_FEATS = 2
TABLE_SIZE = 524288  # 2**19
RESOLUTIONS = [16, 23, 32, 45, 64, 91, 128, 181, 256, 362, 512, 724, 1024,
               1448, 2048, 2896]
PRIMES = (1, 2654435761, 805459861)
BATCH = 2_097_152
N_CORES = 8

P = 128
BC = BATCH // N_CORES          # 262144 elements per core
SPP = BC // P                  # 2048 elements per partition
S_CHUNK = 128                  # s-range per processing chunk
N_CHUNKS = SPP // S_CHUNK      # 16 chunks per level
NI = 16 * S_CHUNK              # ap_gather num_idxs per core per chunk (2048)
SLICE = 32768                  # table entries per partition slice
SCALE = float(2 ** 13)         # table scaling for fp16 storage
MASK19 = 0x7FFFF

K1 = PRIMES[1] & MASK19        # 489905
K2 = PRIMES[2] & MASK19        # 95765

LAST_EXEC_SECONDS = None
LAST_EXEC_NS = None
LAST_TRACE = None


def _emit_floor(nc, pool, src, R, out_dtype, tag, S):
    """g = floor(src * R) for src f32 [P, S]; exact w.r.t. f32 product."""
    v = pool.tile([P, S], dt.float32, tag="fl_v")
    nc.vector.tensor_scalar(v[:], src[:], float(R), None, AluOp.mult)
    r_i = pool.tile([P, S], dt.int32, tag="fl_ri")
    nc.vector.tensor_copy(r_i[:], v[:])          # round-to-nearest
    r_f = pool.tile([P, S], dt.float32, tag="fl_rf")
    nc.vector.tensor_copy(r_f[:], r_i[:])
    c = pool.tile([P, S], dt.float32, tag="fl_c")
    nc.vector.tensor_tensor(c[:], v[:], r_f[:], AluOp.is_lt)  # v < r_f -> 1.0
    g = pool.tile([P, S], out_dtype, tag=tag)
    nc.vector.tensor_tensor(g[:], r_f[:], c[:], AluOp.subtract)
    return g


def _emit_prime_mul(nc, pool, g_f, K, tag, S):
    """int32 tile whose low 19 bits equal (g*K) mod 2^19 (g < 4096)."""
    Khi, Klo = K >> 7, K & 127
    a = pool.tile([P, S], dt.int32, tag="pm_a")
    nc.vector.tensor_scalar(a[:], g_f[:], float(Khi), None, AluOp.mult)
    b = pool.tile([P, S], dt.int32, tag="pm_b")
    nc.vector.tensor_scalar(b[:], g_f[:], float(Klo), None, AluOp.mult)
    a0 = pool.tile([P, S], dt.int32, tag="pm_a0")
    nc.vector.tensor_scalar(a0[:], a[:], 0xFFF, None, AluOp.bitwise_and)
    comb = pool.tile([P, S], dt.int32, tag=tag)
    nc.vector.scalar_tensor_tensor(comb[:], a0[:], 128.0, b[:], AluOp.mult,
                                   AluOp.add)
    return comb


def build_nc():
    nc = bacc.Bacc(None, target_bir_lowering=False)

    # Per-core inputs: coords [3, P, SPP] f32, replicated tables
    # [N_LEVELS, P, SLICE, 2] f16 (pre-scaled by SCALE), block-diag ones.
    coords_in = nc.dram_tensor("coords3", [3, P, SPP], dt.float32,
                               kind="ExternalInput")
    tblr = nc.dram_tensor("tblr", [N_LEVELS, P, SLICE, N_FEATS], dt.float16,
                          kind="ExternalInput")
    b16_in = nc.dram_tensor("b16", [P, 8], dt.float16, kind="ExternalInput")
    wq_in = nc.dram_tensor("wq", [P, 16, S_CHUNK], dt.float16,
                           kind="ExternalInput")
    sid_in = nc.dram_tensor("sid", [P, 1], dt.float32, kind="ExternalInput")
    out = nc.dram_tensor("out", [N_LEVELS, N_CHUNKS, 8, NI * N_FEATS],
                         dt.float32, kind="ExternalOutput")

    with tile.TileContext(nc) as tc:
        with (
            tc.tile_pool(name="tabp", bufs=1) as tabp,
            tc.tile_pool(name="workp", bufs=2) as workp,
            tc.tile_pool(name="hashp", bufs=1) as hashp,
            tc.tile_pool(name="iop", bufs=2) as iop,
            tc.tile_pool(name="coop", bufs=2) as coop,
            tc.tile_pool(name="constp", bufs=1) as constp,
            tc.tile_pool(name="psump", bufs=4, space="PSUM") as psump,
            tc.tile_pool(name="psumq", bufs=4, space="PSUM") as psumq,
        ):
            b16 = constp.tile([P, 8], dt.float16, tag="b16")
            nc.sync.dma_start(b16[:], b16_in[:])
            wq = constp.tile([P, 16, S_CHUNK], dt.float16, tag="wq")
            nc.sync.dma_start(wq[:], wq_in[:])
            sid = constp.tile([P, 1], dt.float32, tag="sid")
            nc.sync.dma_start(sid[:], sid_in[:])
            mask19t = constp.tile([P, 1], dt.int32, tag="mask19t")
            nc.vector.memset(mask19t[:], MASK19)

            tabt = tabp.tile([P, SLICE, N_FEATS], dt.float16, tag="tabt")

            def emit_front(lvl, ch):
                """coords load + hash + candidate-gather issue."""
                R = RESOLUTIONS[lvl]
                s0 = ch * S_CHUNK
                sl = slice(s0, s0 + S_CHUNK)
                xt = coop.tile([P, S_CHUNK], dt.float32, tag="xt")
                yt = coop.tile([P, S_CHUNK], dt.float32, tag="yt")
                zt = coop.tile([P, S_CHUNK], dt.float32, tag="zt")
                nc.sync.dma_start(xt[:], coords_in[0, :, sl])
                nc.sync.dma_start(yt[:], coords_in[1, :, sl])
                nc.sync.dma_start(zt[:], coords_in[2, :, sl])

                gx = _emit_floor(nc, hashp, xt, R, dt.int32, "gx", S_CHUNK)
                gy = _emit_floor(nc, hashp, yt, R, dt.float32, "gy", S_CHUNK)
                gz = _emit_floor(nc, hashp, zt, R, dt.float32, "gz", S_CHUNK)
                py_ = _emit_prime_mul(nc, hashp, gy, K1, "py", S_CHUNK)
                pz_ = _emit_prime_mul(nc, hashp, gz, K2, "pz", S_CHUNK)
                # scratch tags pm_a/pm_b/pm_a0 are dead by now; alias them
                t1 = hashp.tile([P, S_CHUNK], dt.int32, tag="pm_a")
                nc.vector.scalar_tensor_tensor(
                    t1[:], py_[:], mask19t[:], gx[:],
                    AluOp.bitwise_and, AluOp.bitwise_xor)
                h = hashp.tile([P, S_CHUNK], dt.int32, tag="pm_b")
                nc.vector.scalar_tensor_tensor(
                    h[:], pz_[:], mask19t[:], t1[:],
                    AluOp.bitwise_and, AluOp.bitwise_xor)
                lo32 = hashp.tile([P, S_CHUNK], dt.int32, tag="pm_a0")
                nc.vector.tensor_scalar(lo32[:], h[:], 0x7FFF, None,
                                        AluOp.bitwise_and)
                lo = iop.tile([P, S_CHUNK], dt.int16, tag="lo")
                nc.vector.tensor_copy(lo[:], lo32[:])
                hi32 = hashp.tile([P, S_CHUNK], dt.int32, tag="pm_a")
                nc.vector.tensor_scalar(hi32[:], h[:], 15, None,
                                        AluOp.logical_shift_right)
                hi_f = iop.tile([P, S_CHUNK], dt.float16, tag="hi_f")
                nc.vector.tensor_copy(hi_f[:], hi32[:])

                cand = workp.tile([P, NI, N_FEATS], dt.float16, tag="cand")
                nc.gpsimd.ap_gather(cand[:], tabt[:], lo[:], channels=P,
                                    num_elems=SLICE, d=N_FEATS,
                                    num_idxs=NI)
                return lvl, ch, cand, hi_f

            def emit_back(state):
                """slice-mask build + select + store for a front-stage chunk."""
                lvl, ch, cand, hi_f = state
                # broadcast hi across each 16-partition group via selector
                # matmuls on PE, compare against the partition's slice id
                maskt = workp.tile([P, NI, N_FEATS], dt.float16, tag="maskt")
                for q in range(16):
                    psq = psumq.tile([P, S_CHUNK], dt.float32,
                                     space="PSUM", tag="psq")
                    nc.tensor.matmul(psq[:], wq[:, q], hi_f[:],
                                     start=True, stop=True)
                    psb = psq[:].unsqueeze(2).broadcast_to(
                        [P, S_CHUNK, N_FEATS])
                    nc.vector.tensor_scalar(
                        maskt[:, q::16, :], psb, sid[:], None,
                        AluOp.is_equal)

                # mask-mult in place, block-sum on PE, descale on evacuation
                cfl = cand[:].rearrange("p n f -> p (n f)")
                mfl = maskt[:].rearrange("p n f -> p (n f)")
                nc.vector.tensor_tensor(cfl, cfl, mfl, AluOp.mult)
                sel = workp.tile([8, NI * N_FEATS], dt.float32, tag="sel")
                NCOL = 512
                for mcol in range(0, NI * N_FEATS, NCOL):
                    ps = psump.tile([8, NCOL], dt.float32, space="PSUM",
                                    tag="ps")
                    nc.tensor.matmul(ps[:], b16[:],
                                     cfl[:, mcol:mcol + NCOL],
                                     start=True, stop=True)
                    nc.vector.tensor_scalar(sel[:, mcol:mcol + NCOL],
                                            ps[:], 1.0 / SCALE,
                                            None, AluOp.mult)
                nc.sync.dma_start(out[lvl, ch], sel[:])

            # one-stage software pipeline: next chunk's hash+gather is emitted
            # (and thus sequenced on DVE/Pool) ahead of the current chunk's
            # select stage, so gathers run back-to-back
            pending = None
            for lvl in range(N_LEVELS):
                nc.sync.dma_start(tabt[:], tblr[lvl])
                for ch in range(N_CHUNKS):
                    front = emit_front(lvl, ch)
                    if pending is not None:
                        emit_back(pending)
                    pending = front
            emit_back(pending)

    nc.compile()
    return nc


def _prep_tables(tables):
    """[L, T, F] f32 -> replicated sliced fp16 [L, P, SLICE, F] (scaled)."""
    t16 = (tables * SCALE).astype(np.float16)  # [L, T, F]
    sl = t16.reshape(N_LEVELS, 16, SLICE, N_FEATS)  # slice s = entries s*SLICE+
    # partition p holds slice p % 16
    return np.ascontiguousarray(sl[:, np.arange(P) % 16])  # [L, P, SLICE, F]


def kernel(coords, tables):
    global LAST_EXEC_SECONDS
    coords = np.asarray(coords, dtype=np.float32)
    tables = np.asarray(tables, dtype=np.float32)

    tblr = _prep_tables(tables)
    b16 = np.zeros((P, 8), np.float16)
    for g in range(8):
        b16[g * 16:(g + 1) * 16, g] = 1.0
    # selector weights: wq[p, q, c] = 1 iff p == (c//16)*16 + q
    wq = np.zeros((P, 16, S_CHUNK), np.float16)
    for q in range(16):
        for c in range(S_CHUNK):
            wq[(c // 16) * 16 + q, q, c] = 1.0
    sid = (np.arange(P) % 16).astype(np.float32).reshape(P, 1)

    nc = build_nc()

    in_maps = []
    for c in range(N_CORES):
        csl = coords[c * BC:(c + 1) * BC]  # [BC, 3]
        c3 = np.ascontiguousarray(csl.T.reshape(3, P, SPP))
        in_maps.append({"coords3": c3, "tblr": tblr, "b16": b16, "wq": wq,
                        "sid": sid})

    import time
    global LAST_EXEC_NS, LAST_TRACE
    t0 = time.time()
    res = run_bass_kernel_spmd(nc, in_maps, core_ids=list(range(N_CORES)))
    LAST_EXEC_SECONDS = time.time() - t0
    LAST_EXEC_NS = getattr(res, "exec_time_ns", None)
    LAST_TRACE = getattr(res, "instructions_and_trace", None)

    out = np.empty((BATCH, N_LEVELS * N_FEATS), np.float32)
    n_lvl = len(RESOLUTIONS[:N_LEVELS])
    for c in range(N_CORES):
        oc = res.results[c]["out"]  # [L, NCH, 8, NI*F]
        oc = oc.reshape(n_lvl, N_CHUNKS, 8, S_CHUNK, 16, N_FEATS)
        # axes (l, ch, g, sj, q, f) -> b = ((g*16+q)*SPP + ch*S_CHUNK + sj)
        oc = oc.transpose(2, 4, 1, 3, 0, 5).reshape(BC, n_lvl * N_FEATS)
        out[c * BC:(c + 1) * BC] = oc
    return out

